# revision 15
# baseline (speedup 1.0000x reference)
"""Trainium2 Bass kernel for nn_BlockAttnRes (block-softmax residual net).

Shapes: embedding [8, 8192, 128] f32, L=16 layers, BLOCK_SIZE=4.
Sharding: batch dim B=8 across 8 cores (1 batch row / core = 8192 tokens).

Per-core: tokens-on-partitions ("row") bf16 state resident in SBUF.
5 row slots: slot0 = emb, slot 1+g = partial of group g (becomes block g+1
at commit). For_i over token tiles (F=512 tokens = 4 blocks of 128),
python-unrolled 16 layers inside, NS=4 tiles interleaved per iteration.

Design (v10, from v9 ~2.31 ms):
  - v9 was DVE-bound (85% busy, 11k instrs, overhead-dominated FD=128 ops).
  - slot sum-of-squares now via ACT Square(PSUM partial) -> psq_col, plus a
    PE ones-matmul row appended to the stats band; the existing stats
    transpose carries ssq to row layout for free (was ~1000 DVE ops).
  - weighted sum: product/add tree at FD=512 (stride-0 broadcast of wts
    along d) instead of 4n per-block FMA chains; products alternate
    DVE/GpSimd (Pool engine was idle), adds on DVE.
  - hsum (LN mean numerator) recovered from per-source rowsum stats:
    hsum = sum_i E_i * rs_i (2 tiny ops) since TT ops can't ride accums.
  - reciprocal -> reciprocal_approx_fast (~5x).
  - emb fed from host pre-cast bf16 in BOTH row layout and transposed
    column layout (direct DMA; kills per-stream PE transposes + copies).
  - all state bf16; partial accumulated in COLUMN layout by the W2 matmuls
    (PSUM f32 across the 4-layer group)
  - softmax normalizer 1/den FOLDED into LayerNorm for l<15:
    LN(u/den) == (u - mu_u) * rsqrt(var_u + eps*den^2) exactly
  - softmax-exp via tanh identity e^t=(1+T)/(1-T) (gelu ACT table only)
  - rsqrt via int bit-trick seed + Newton iterations (DVE only)
  - LayerNorm affine folded into W1' = diag(g)@W1, b1' = b1 + ln_b@W1
  - PYTHONHASHSEED pinned for neuronxcc subprocesses
"""
import contextlib
import ctypes
import os
import sys
import types
from contextlib import ExitStack

os.environ.setdefault("PYTHONHASHSEED", "1")

sys.path.insert(0, "/opt/trn_rl_repo")


def _install_ntff_hook():
    """Provide antenv.axon_hooks (missing in the trimmed repo) so
    run_bass_kernel_spmd(trace=True) can collect NTFF profiles."""
    if "antenv.axon_hooks" in sys.modules:
        return
    try:
        lib = ctypes.CDLL("/opt/axon/libaxon_pjrt.so")
    except OSError:
        return
    if not hasattr(lib, "axon_start_nrt_profile"):
        hook = None
    else:
        lib.axon_start_nrt_profile.argtypes = [
            ctypes.POINTER(ctypes.c_int64), ctypes.c_size_t]
        lib.axon_start_nrt_profile.restype = ctypes.c_int64
        lib.axon_stop_nrt_profile.argtypes = [ctypes.c_char_p]
        lib.axon_stop_nrt_profile.restype = ctypes.c_int64

        @contextlib.contextmanager
        def hook(output_dir, device_ids):
            import jax
            jax.devices()
            if device_ids:
                ids = (ctypes.c_int64 * len(device_ids))(*device_ids)
                rc = lib.axon_start_nrt_profile(ids, len(device_ids))
            else:
                rc = lib.axon_start_nrt_profile(None, 0)
            if rc != 0:
                raise RuntimeError(f"axon_start_nrt_profile rc={rc}")
            try:
                yield
            finally:
                n = lib.axon_stop_nrt_profile(str(output_dir).encode())
                print(f"profile: {n} file(s) -> {output_dir}", file=sys.stderr)

    mod = types.ModuleType("antenv.axon_hooks")
    mod.get_axon_ntff_profile_hook = lambda: hook
    mod.set_axon_ntff_profile_hook = lambda h: None
    sys.modules["antenv.axon_hooks"] = mod

import numpy as np
import ml_dtypes

import concourse.bacc as bacc
import concourse.bass as bass
import concourse.mybir as mybir
from concourse.bass_utils import run_bass_kernel_spmd
from concourse.tile import TileContext
from concourse.masks import make_identity

F32 = mybir.dt.float32
BF16 = mybir.dt.bfloat16
I32 = mybir.dt.int32
ALU = mybir.AluOpType
AF = mybir.ActivationFunctionType
AX = mybir.AxisListType

L = 16
GROUP = 4
D = 128
NBLK = 4                 # 128-token blocks per tile
F = NBLK * 128           # tokens per tile
EPS_RMS = 1e-8
EPS_LN = 1e-5
MAGIC = 0x5F3759DF
N_CORES = 8

_CACHE = {}


def _mkap(base, extra_off, dims):
    """Build an AP from base AP's tensor with partition dim kept and given
    free dims [[stride, count], ...] (element units)."""
    return bass.AP(tensor=base.tensor, offset=base.offset + extra_off,
                   ap=[base.ap[0]] + [list(d) for d in dims])


def _bcast(ap, n):
    """Append a stride-0 inner free dim of size n to an AP."""
    return bass.AP(tensor=ap.tensor, offset=ap.offset,
                   ap=list(ap.ap) + [[0, n]])


def _newton_rsqrt(nc, pool, x, shape, iters=2):
    """y = rsqrt(x) for x [128, *shape] f32 tile (positive). Returns y tile."""
    y = pool.tile([128] + list(shape), F32, tag="nw_y", name="nw_y")
    xi = x.bitcast(I32)
    yi = y.bitcast(I32)
    nc.vector.tensor_scalar(out=yi[:], in0=xi[:], scalar1=1, scalar2=0,
                            op0=ALU.logical_shift_right,
                            op1=ALU.logical_shift_right)
    nc.vector.tensor_scalar(out=yi[:], in0=yi[:], scalar1=-1, scalar2=MAGIC,
                            op0=ALU.mult, op1=ALU.add)
    t = pool.tile([128] + list(shape), F32, tag="nw_t", name="nw_t")
    for _ in range(iters):
        nc.vector.tensor_mul(t[:], y[:], y[:])
        nc.vector.scalar_tensor_tensor(out=t[:], in0=t[:], scalar=-0.5,
                                       in1=x[:], op0=ALU.mult, op1=ALU.mult)
        nc.vector.scalar_tensor_tensor(out=y[:], in0=t[:], scalar=1.5,
                                       in1=y[:], op0=ALU.add, op1=ALU.mult)
    return y


def build(tiles_per_core=16):
    nc = bacc.Bacc("TRN2", target_bir_lowering=False)
    n_tok = tiles_per_core * F

    embr = nc.dram_tensor("embr", [n_tok, D], BF16, kind="ExternalInput")
    embc = nc.dram_tensor("embc", [D, n_tok], BF16, kind="ExternalInput")
    # wallT1: col l (l<16) = w[l]; col 16 = ones; col 17 = ones (ssq matmul)
    wallT1 = nc.dram_tensor("wallT1", [D, L + 2], BF16, kind="ExternalInput")
    w1p = nc.dram_tensor("w1p", [D, L * 2 * 128], BF16, kind="ExternalInput")
    b1p = nc.dram_tensor("b1p", [128, 2 * L], F32, kind="ExternalInput")
    w2p = nc.dram_tensor("w2p", [128, L * 2 * D], BF16, kind="ExternalInput")
    out = nc.dram_tensor("out", [n_tok, D], F32, kind="ExternalOutput")

    embr_v = embr.rearrange("(T b p) d -> T p b d", b=NBLK, p=128)
    out_v = out.rearrange("(T b p) d -> T p b d", b=NBLK, p=128)

    NS = 4 if tiles_per_core % 4 == 0 else (
        2 if tiles_per_core % 2 == 0 else 1)

    with TileContext(nc) as tc, ExitStack() as es:
        cst = es.enter_context(tc.tile_pool(name="cst", bufs=1))
        identb = cst.tile([128, 128], BF16)
        make_identity(nc, identb[:])
        wallT1_sb = cst.tile([128, L + 2], BF16)
        nc.sync.dma_start(wallT1_sb[:], wallT1[:])
        w1p_sb = cst.tile([128, L, 2, 128], BF16)
        nc.sync.dma_start(w1p_sb[:], w1p[:].rearrange(
            "d (l h m) -> d l h m", l=L, h=2))
        b1p_sb = cst.tile([128, 2 * L], F32)
        nc.sync.dma_start(b1p_sb[:], b1p[:])
        w2p_sb = cst.tile([128, L, 2, D], BF16)
        nc.sync.dma_start(w2p_sb[:], w2p[:].rearrange(
            "m (l k d) -> m l k d", l=L, k=2))

        sp = es.enter_context(tc.tile_pool(name="state", bufs=NS + 1))
        big = es.enter_context(tc.tile_pool(name="big", bufs=NS + 2))
        sml = es.enter_context(tc.tile_pool(name="sml", bufs=12))
        nwp = es.enter_context(tc.tile_pool(name="nw", bufs=12))
        spd = es.enter_context(tc.tile_pool(name="spd", bufs=1))
        pp_par = es.enter_context(tc.tile_pool(name="pp_par", bufs=NS,
                                               space="PSUM"))
        pp_h1 = es.enter_context(tc.tile_pool(name="pp_h1", bufs=1,
                                              space="PSUM"))
        pp_mm = es.enter_context(tc.tile_pool(name="pp_mm", bufs=1,
                                              space="PSUM"))
        pp_xt = es.enter_context(tc.tile_pool(name="pp_xt", bufs=1,
                                              space="PSUM"))

        def stat_matmuls(st, src_col, psq_col, wcols, nwide, mm_ps):
            """Stream k's 32-partition band of mm_ps:
            1) ssq matmul FIRST: nwide-col stationary ending in the ones
               column writes rows [b0 : b0+nwide); only row b0+nwide-1
               (= sum of psq = ssq) survives, since
            2) dots matmul then overwrites rows [b0 : b0+nwide-1) with
               wcols.T @ src_col (dots / rowsum)."""
            b0 = 32 * st["k"]
            wv = wallT1_sb[:]
            sel = bass.AP(tensor=wv.tensor, offset=wv.offset + (18 - nwide),
                          ap=[wv.ap[0], [1, nwide]])
            nc.tensor.matmul(mm_ps[b0:b0 + nwide, :], sel,
                             psq_col[:], start=True, stop=True,
                             tile_position=(0, b0),
                             skip_group_check=True)
            nrow = wcols.ap[-1][1]
            nc.tensor.matmul(mm_ps[b0:b0 + nrow, :], wcols,
                             src_col[:], start=True, stop=True,
                             tile_position=(0, b0),
                             skip_group_check=True)

        def transpose_stats(mm_ps, tag):
            """Stats -> row layout [128, NBLK, 128] f32 (lane 32k+j = stat_j
            of stream k)."""
            mm_sb = big.tile([128, F], BF16, tag="mm_sb", name="mm_sb")
            nc.scalar.copy(mm_sb[:], mm_ps[:])
            tt_ps = pp_xt.tile([128, F], BF16, tag="xt", name="tt_ps")
            for c in range(NBLK):
                nc.tensor.matmul(tt_ps[:, c * 128:(c + 1) * 128],
                                 mm_sb[:, c * 128:(c + 1) * 128],
                                 identb[:],
                                 is_transpose=True, start=True, stop=True,
                                 skip_group_check=True)
            row = sml.tile([128, NBLK, 128], F32, tag=tag, name=tag,
                           bufs=3)
            nc.vector.tensor_copy(
                _mkap(row[:], 0, [[1, F]]), tt_ps[:])
            return row

        def creation_finish(sts, s_idx, stats_row, sh):
            """stats_row [128, NBLK, 128] (lane 32k+j = dot_j, 32k+16 =
            rowsum, 32k+17 = ssq) -> scaled sdots_all[:, :, s_idx] and
            rs_all[:, :, s_idx]."""
            ns_ = len(sts)
            sa = sh["sdots_all"][:]
            sr = stats_row[:]
            xs = sml.tile([128, NS, NBLK], F32, tag="xs_cr", name="xs_cr")
            nc.vector.tensor_scalar(
                out=xs[:, 0:ns_],
                in0=_mkap(sr, 17, [[32, ns_], [128, NBLK]]),
                scalar1=1.0 / D, scalar2=EPS_RMS,
                op0=ALU.mult, op1=ALU.add)
            rms = _newton_rsqrt(nc, nwp, xs, (NS, NBLK))
            r_ap = rms[:]
            for k in range(ns_):
                # out: sdots_all slice (blk, l) at (k, s_idx)
                nc.vector.scalar_tensor_tensor(
                    out=_mkap(sa, k * 5 * NBLK * L + s_idx * NBLK * L,
                              [[L, NBLK], [1, L]]),
                    in0=_mkap(sr, 32 * k, [[128, NBLK], [1, L]]),
                    scalar=1.0,
                    in1=_mkap(r_ap, k * NBLK, [[1, NBLK], [0, L]]),
                    op0=ALU.bypass, op1=ALU.mult)

        def tile_start(it, k, sh):
            st = {"it": it, "k": k, "sh": sh}
            st["slots"] = sp.tile([128, 5, NBLK, D], BF16, tag="slots",
                                  name="slots")
            st["trash"] = sp.tile([128, NBLK, D], BF16, tag="trash",
                                  name="trash")
            st["partial_ps"] = pp_par.tile([128, F], F32, tag="par",
                                           name="par")
            # emb bf16 row layout (host pre-cast)
            nc.sync.dma_start(out=st["slots"][:, 0], in_=embr_v[bass.ds(it, 1)])
            # emb bf16 column layout (host pre-transposed)
            ecol = big.tile([128, F], BF16, tag="ecol", name="ecol")
            nc.sync.dma_start(out=ecol[:],
                              in_=embc[:, bass.ds(it * F, F)])
            st["ecol"] = ecol
            esq = big.tile([128, F], BF16, tag="psq", name="esq")
            nc.scalar.activation(out=esq[:], in_=ecol[:], func=AF.Square)
            st["esq"] = esq
            return st

        def emit_layer(sts, l, sh):
            ns_ = len(sts)
            g, j = l // GROUP, l % GROUP
            nsrc = g + 1
            has_p = j > 0
            n = nsrc + (1 if has_p else 0)
            last = l == L - 1
            sdots_all = sh["sdots_all"]

            # source-outermost layout [128, 5, NS, NBLK] so source slices
            # collapse to [128, n, NS*NBLK] (custom-DVE rank limit)
            SEG = NS * NBLK
            E_T = sml.tile([128, 5, NS, NBLK], F32, tag="E_T", name="E_T")
            e_ap = E_T[:]
            statics_out = bass.AP(
                tensor=e_ap.tensor, offset=e_ap.offset,
                ap=[e_ap.ap[0], [NBLK, ns_], [SEG, nsrc], [1, NBLK]])
            nc.scalar.activation(out=statics_out,
                                 in_=sdots_all[:, 0:ns_, 0:nsrc, :, l],
                                 func=AF.Tanh, scale=0.5)

            if has_p:
                pr = sh["pstat_row"][:]
                # pstat_row lanes: 32k+0 = dot, 32k+1 = rowsum, 32k+2 = ssq
                xp = sml.tile([128, NS, NBLK], F32, tag="xp", name="xp")
                nc.vector.tensor_scalar(
                    out=xp[:, 0:ns_],
                    in0=_mkap(pr, 2, [[32, ns_], [128, NBLK]]),
                    scalar1=1.0 / D, scalar2=EPS_RMS,
                    op0=ALU.mult, op1=ALU.add)
                rmsp = _newton_rsqrt(nc, nwp, xp, (NS, NBLK), iters=1)
                lp = sml.tile([128, NS, NBLK], F32, tag="lp", name="lp")
                nc.vector.tensor_mul(
                    lp[:, 0:ns_],
                    _mkap(pr, 0, [[32, ns_], [128, NBLK]]),
                    rmsp[:, 0:ns_])
                nc.scalar.activation(out=E_T[:, nsrc, 0:ns_, :],
                                     in_=lp[:, 0:ns_],
                                     func=AF.Tanh, scale=0.5)

            def seg(t, nn):
                # [128, nn, SEG] collapsed view of a [128, 5, NS, NBLK] tile
                return _mkap(t[:], 0, [[SEG, nn], [1, SEG]])

            Ev = seg(E_T, n)
            Bt = sml.tile([128, 5, NS, NBLK], F32, tag="B", name="Bt")
            nc.vector.tensor_scalar(out=seg(Bt, n), in0=Ev,
                                    scalar1=-1.0, scalar2=-1.0,
                                    op0=ALU.mult, op1=ALU.subtract)
            R = sml.tile([128, 5, NS, NBLK], F32, tag="R", name="R")
            nc.vector.reciprocal_approx_fast(seg(R, n), seg(Bt, n))
            E = sml.tile([128, 5, NS, NBLK], F32, tag="E", name="E")
            nc.vector.tensor_scalar(out=seg(E, n), in0=seg(R, n),
                                    scalar1=2.0, scalar2=-1.0,
                                    op0=ALU.mult, op1=ALU.add)
            den = sml.tile([128, NS, NBLK], F32, tag="den", name="den")
            # reduce over the (outer) source dim via AP reordering
            nc.vector.tensor_reduce(den[:, 0:ns_],
                                    _mkap(E[:], 0, [[1, SEG], [SEG, n]]),
                                    axis=AX.X, op=ALU.add)
            if last:
                # final output must be normalized: wts = E / den
                rd = sml.tile([128, NS, NBLK], F32, tag="rd", name="rd")
                nc.vector.reciprocal_approx_fast(rd[:, 0:ns_], den[:, 0:ns_])
                wts = sml.tile([128, 5, NS, NBLK], F32, tag="wts",
                               name="wts")
                nc.vector.scalar_tensor_tensor(
                    out=seg(wts, n), in0=seg(E, n),
                    scalar=1.0,
                    in1=_mkap(rd[:], 0, [[0, n], [1, SEG]]),
                    op0=ALU.bypass, op1=ALU.mult)
            else:
                # unnormalized u = sum_i E_i V_i; the 1/den normalizer is
                # folded into LayerNorm: LN(u/den) = (u - mu_u) *
                # rsqrt(var_u + eps*den^2), exactly
                wts = E

            def wsc(k, blk, i):
                # per-partition scalar AP: wts for (stream k, blk, source i)
                wv = wts[:]
                return bass.AP(tensor=wv.tensor,
                               offset=wv.offset + i * SEG + k * NBLK + blk,
                               ap=[wv.ap[0], [1, 1]])

            # weighted sum: per-block fused mult-add stt chains (DVE),
            # hsum riding the last op's hardware accumulator
            hsum = sml.tile([128, NS, NBLK], F32, tag="hsum", name="hsum")
            hs = []
            for st in sts:
                k = st["k"]
                slots = st["slots"]
                h = big.tile([128, NBLK, D], F32 if last else BF16,
                             tag="h_f32" if last else "h", name="h")
                hs.append(h)
                for blk in range(NBLK):
                    acc = hsum[:, k, blk:blk + 1] if not last else None
                    if n == 1:
                        nc.vector.tensor_scalar(
                            out=h[:, blk, :], in0=slots[:, 0, blk, :],
                            scalar1=wsc(k, blk, 0), scalar2=0.0,
                            op0=ALU.mult, op1=ALU.add, accum_out=acc)
                    else:
                        nc.vector.tensor_scalar(
                            out=h[:, blk, :], in0=slots[:, 0, blk, :],
                            scalar1=wsc(k, blk, 0), scalar2=None,
                            op0=ALU.mult)
                    for i in range(1, n):
                        nc.vector.scalar_tensor_tensor(
                            out=h[:, blk, :], in0=slots[:, i, blk, :],
                            scalar=wsc(k, blk, i),
                            in1=h[:, blk, :],
                            op0=ALU.mult, op1=ALU.add,
                            accum_out=(acc if i == n - 1 else None))
                if last:
                    nc.sync.dma_start(out=out_v[bass.ds(st["it"], 1)],
                                      in_=h[:])
            if last:
                return

            # hssq per block with accumulator rides (trash outputs)
            hssq = sml.tile([128, NS, NBLK], F32, tag="hssq", name="hssq")
            for st, h in zip(sts, hs):
                k = st["k"]
                for blk in range(NBLK):
                    nc.vector.scalar_tensor_tensor(
                        out=st["trash"][:, blk, :], in0=h[:, blk, :],
                        scalar=1.0, in1=h[:, blk, :],
                        op0=ALU.bypass, op1=ALU.mult,
                        accum_out=hssq[:, k, blk:blk + 1])

            m2 = sml.tile([128, NS, NBLK], F32, tag="m2", name="m2")
            nc.vector.tensor_mul(m2[:, 0:ns_], hsum[:, 0:ns_], hsum[:, 0:ns_])
            den2e = sml.tile([128, NS, NBLK], F32, tag="den2e",
                             name="den2e")
            nc.vector.scalar_tensor_tensor(
                out=den2e[:, 0:ns_], in0=den[:, 0:ns_], scalar=EPS_LN,
                in1=den[:, 0:ns_], op0=ALU.mult, op1=ALU.mult)
            t1 = sml.tile([128, NS, NBLK], F32, tag="t1", name="t1")
            nc.vector.scalar_tensor_tensor(
                out=t1[:, 0:ns_], in0=hssq[:, 0:ns_], scalar=1.0 / D,
                in1=den2e[:, 0:ns_], op0=ALU.mult, op1=ALU.add)
            xs2 = sml.tile([128, NS, NBLK], F32, tag="xs2", name="xs2")
            nc.vector.scalar_tensor_tensor(
                out=xs2[:, 0:ns_], in0=m2[:, 0:ns_], scalar=-1.0 / (D * D),
                in1=t1[:, 0:ns_], op0=ALU.mult, op1=ALU.add)
            s_ln = _newton_rsqrt(nc, nwp, xs2, (NS, NBLK), iters=1)
            mu = sml.tile([128, NS, NBLK], F32, tag="mu", name="mu")
            nc.vector.tensor_scalar_mul(mu[:, 0:ns_], hsum[:, 0:ns_], 1.0 / D)

            # xn = (h - mu) * s per block (fused dual-scalar ts)
            for st, h in zip(sts, hs):
                k = st["k"]
                xn = big.tile([128, NBLK, D], BF16, tag="xn", name="xn")
                for blk in range(NBLK):
                    nc.vector.tensor_scalar(
                        out=xn[:, blk, :], in0=h[:, blk, :],
                        scalar1=mu[:, k, blk:blk + 1],
                        scalar2=s_ln[:, k, blk:blk + 1],
                        op0=ALU.subtract, op1=ALU.mult)
                xnT_ps = pp_xt.tile([128, F], BF16, tag="xt", name="xnT_ps")
                for blk in range(NBLK):
                    nc.tensor.matmul(xnT_ps[:, blk * 128:(blk + 1) * 128],
                                     xn[:, blk, :], identb[:],
                                     is_transpose=True, start=True, stop=True,
                                     skip_group_check=True)
                xn_col = big.tile([128, F], BF16, tag="xn_col", name="xn_col")
                nc.vector.tensor_copy(xn_col[:], xnT_ps[:])
                st["xn_col"] = xn_col

            # MLP: W1 -> gelu -> W2 accumulating into column partial PSUM
            # (h1 double-buffered so consecutive W1 matmuls overlap gelus)
            for st in sts:
                G = []
                for half in range(2):
                    h1 = pp_h1.tile([128, F], F32, tag="h1", name="h1",
                                    bufs=2)
                    nc.tensor.matmul(h1[:], w1p_sb[:, l, half, :],
                                     st["xn_col"][:], start=True, stop=True,
                                     skip_group_check=True)
                    gh = big.tile([128, F], BF16, tag=f"g{half}", name="gh")
                    nc.scalar.activation(
                        gh[:], h1[:], AF.Gelu,
                        bias=b1p_sb[:, 2 * l + half:2 * l + half + 1])
                    G.append(gh)
                for kh in range(2):
                    nc.tensor.matmul(
                        st["partial_ps"][:], w2p_sb[:, l, kh, :], G[kh][:],
                        start=(j == 0 and kh == 0),
                        stop=((j == GROUP - 1 or l == L - 2) and kh == 1),
                        skip_group_check=True)
                pcol = big.tile([128, F], BF16, tag="pcol", name="pcol")
                nc.vector.tensor_copy(pcol[:], st["partial_ps"][:])
                st["pcol"] = pcol
                # squared partial for the ssq stats row (ACT, SBUF src to
                # avoid PSUM port contention with the pcol copy)
                psq = big.tile([128, F], BF16, tag="psq", name="psq")
                nc.scalar.activation(out=psq[:], in_=pcol[:],
                                     func=AF.Square)
                st["psq"] = psq
                # partial row slot via PE transpose + DVE copy
                prow_ps = pp_xt.tile([128, F], BF16, tag="xt",
                                     name="prow_ps")
                for blk in range(NBLK):
                    nc.tensor.matmul(prow_ps[:, blk * 128:(blk + 1) * 128],
                                     pcol[:, blk * 128:(blk + 1) * 128],
                                     identb[:],
                                     is_transpose=True, start=True, stop=True,
                                     skip_group_check=True)
                nc.vector.tensor_copy(
                    _mkap(st["slots"][:], (g + 1) * NBLK * D, [[1, F]]),
                    prow_ps[:])

            # stats for next layer (partial) or creation (commit)
            mm_ps = pp_mm.tile([128, F], F32, tag="mm", name="mm_ps")
            if j < GROUP - 1:
                wv = wallT1_sb[:]
                # cols l+1 (dot) and 16 (ones -> rowsum)
                wcols = bass.AP(tensor=wv.tensor, offset=wv.offset + l + 1,
                                ap=[wv.ap[0], [L - (l + 1), 2]])
                for st in sts:
                    stat_matmuls(st, st["pcol"], st["psq"], wcols, 3, mm_ps)
                sh["pstat_row"] = transpose_stats(mm_ps, "pstat_row")
            else:
                for st in sts:
                    stat_matmuls(st, st["pcol"], st["psq"],
                                 wallT1_sb[:, 0:L + 1], 18, mm_ps)
                stats_row = transpose_stats(mm_ps, "stats_row")
                creation_finish(sts, g + 1, stats_row, sh)

        spd_pool = spd
        with tc.For_i(0, tiles_per_core // NS, 1,
              hint_engines=(mybir.EngineType.DVE,
                            mybir.EngineType.Activation,
                            mybir.EngineType.PE,
                            mybir.EngineType.Pool)) as it0:
            sh = {}
            sh["sdots_all"] = spd_pool.tile([128, NS, 5, NBLK, L], F32,
                                            tag="sdots_all", name="sdots_all")
            sts = [tile_start(it0 * NS + k, k, sh) for k in range(NS)]
            # emb creation stats (memset clears stale psum in unused rows so
            # the transpose/selection matmuls never touch NaN garbage)
            mm_ps = pp_mm.tile([128, F], F32, tag="mm", name="mm_ps")
            nc.vector.memset(mm_ps[:], 0.0)
            for st in sts:
                stat_matmuls(st, st["ecol"], st["esq"],
                             wallT1_sb[:, 0:L + 1], 18, mm_ps)
            stats_row = transpose_stats(mm_ps, "stats_row")
            creation_finish(sts, 0, stats_row, sh)
            for l in range(L):
                emit_layer(sts, l, sh)

    nc.finalize()
    return nc


def _prep_consts(w, ln_g, ln_b, W1, b1, W2):
    bf = ml_dtypes.bfloat16
    W1p = ln_g[:, :, None] * W1                                   # diag(g) @ W1
    b1p = b1 + np.einsum("ld,ldm->lm", ln_b, W1)                  # b1 + ln_b @ W1
    w1p = np.ascontiguousarray(W1p.transpose(1, 0, 2)).reshape(D, L * 2 * 128)
    b1p_sb = b1p.reshape(L, 2, 128).transpose(2, 0, 1).reshape(128, 2 * L)
    w2p = W2.reshape(L, 2, 128, D).transpose(2, 0, 1, 3)
    w2p = np.ascontiguousarray(w2p).reshape(128, L * 2 * D)
    wallT1 = np.concatenate([w.T, np.ones((D, 2), np.float32)], axis=1)
    return {
        "wallT1": np.ascontiguousarray(wallT1).astype(bf),
        "w1p": w1p.astype(bf),
        "b1p": np.ascontiguousarray(b1p_sb).astype(np.float32),
        "w2p": w2p.astype(bf),
    }


def kernel(embedding, w, ln_g, ln_b, W1, b1, W2, b2, _tiles=16, _trace=False):
    if _trace:
        _install_ntff_hook()
    B, T, Dd = embedding.shape
    assert Dd == D
    n_tok = _tiles * F

    key = ("k", _tiles)
    if key not in _CACHE:
        _CACHE[key] = build(_tiles)
    nc = _CACHE[key]

    assert np.all(np.asarray(b2) == 0.0), "nonzero b2 unsupported"
    consts = _prep_consts(np.asarray(w, np.float32),
                          np.asarray(ln_g, np.float32),
                          np.asarray(ln_b, np.float32),
                          np.asarray(W1, np.float32),
                          np.asarray(b1, np.float32),
                          np.asarray(W2, np.float32))
    bf = ml_dtypes.bfloat16
    emb_full = np.asarray(embedding, np.float32).reshape(B * T, D)

    per_core = B * T // N_CORES
    in_maps = []
    for c in range(N_CORES):
        shard = emb_full[c * per_core:(c + 1) * per_core][:n_tok]
        shard_bf = shard.astype(bf)
        in_maps.append({"embr": shard_bf,
                        "embc": np.ascontiguousarray(shard_bf.T),
                        **consts})

    res = run_bass_kernel_spmd(nc, in_maps, core_ids=list(range(N_CORES)),
                               trace=_trace)
    outs = [res.results[c]["out"] for c in range(N_CORES)]
    full = np.stack(outs).reshape(N_CORES, n_tok, D)
    kernel.last_exec_ns = getattr(res, "exec_time_ns", None)
    kernel.last_mean_ns = getattr(res, "mean_exec_time_ns", None)
    if n_tok == per_core:
        return full.reshape(B, T, D)
    return full  # debug partial run


# revision 16
# speedup vs baseline: 1.1926x; 1.1926x over previous
"""Trainium2 Bass kernel for nn_BlockAttnRes (block-softmax residual net).

Shapes: embedding [8, 8192, 128] f32, L=16 layers, BLOCK_SIZE=4.
Sharding: batch dim B=8 across 8 cores (1 batch row / core = 8192 tokens).

Per-core: tokens-on-partitions ("row") bf16 state resident in SBUF.
5 row slots: slot0 = emb, slot 1+g = partial of group g (becomes block g+1
at commit). For_i over token tiles (F=512 tokens = 4 blocks of 128),
python-unrolled 16 layers inside, NS=4 tiles interleaved per iteration.

Design (v10, from v9 ~2.31 ms):
  - v9 was DVE-bound (85% busy, 11k instrs, overhead-dominated FD=128 ops).
  - slot sum-of-squares now via ACT Square(PSUM partial) -> psq_col, plus a
    PE ones-matmul row appended to the stats band; the existing stats
    transpose carries ssq to row layout for free (was ~1000 DVE ops).
  - weighted sum: product/add tree at FD=512 (stride-0 broadcast of wts
    along d) instead of 4n per-block FMA chains; products alternate
    DVE/GpSimd (Pool engine was idle), adds on DVE.
  - hsum (LN mean numerator) recovered from per-source rowsum stats:
    hsum = sum_i E_i * rs_i (2 tiny ops) since TT ops can't ride accums.
  - reciprocal -> reciprocal_approx_fast (~5x).
  - emb fed from host pre-cast bf16 in BOTH row layout and transposed
    column layout (direct DMA; kills per-stream PE transposes + copies).
  - all state bf16; partial accumulated in COLUMN layout by the W2 matmuls
    (PSUM f32 across the 4-layer group)
  - softmax normalizer 1/den FOLDED into LayerNorm for l<15:
    LN(u/den) == (u - mu_u) * rsqrt(var_u + eps*den^2) exactly
  - softmax-exp via tanh identity e^t=(1+T)/(1-T) (gelu ACT table only)
  - rsqrt via int bit-trick seed + Newton iterations (DVE only)
  - LayerNorm affine folded into W1' = diag(g)@W1, b1' = b1 + ln_b@W1
  - PYTHONHASHSEED pinned for neuronxcc subprocesses
"""
import contextlib
import ctypes
import os
import sys
import types
from contextlib import ExitStack

os.environ.setdefault("PYTHONHASHSEED", "1")

sys.path.insert(0, "/opt/trn_rl_repo")


def _install_ntff_hook():
    """Provide antenv.axon_hooks (missing in the trimmed repo) so
    run_bass_kernel_spmd(trace=True) can collect NTFF profiles."""
    if "antenv.axon_hooks" in sys.modules:
        return
    try:
        lib = ctypes.CDLL("/opt/axon/libaxon_pjrt.so")
    except OSError:
        return
    if not hasattr(lib, "axon_start_nrt_profile"):
        hook = None
    else:
        lib.axon_start_nrt_profile.argtypes = [
            ctypes.POINTER(ctypes.c_int64), ctypes.c_size_t]
        lib.axon_start_nrt_profile.restype = ctypes.c_int64
        lib.axon_stop_nrt_profile.argtypes = [ctypes.c_char_p]
        lib.axon_stop_nrt_profile.restype = ctypes.c_int64

        @contextlib.contextmanager
        def hook(output_dir, device_ids):
            import jax
            jax.devices()
            if device_ids:
                ids = (ctypes.c_int64 * len(device_ids))(*device_ids)
                rc = lib.axon_start_nrt_profile(ids, len(device_ids))
            else:
                rc = lib.axon_start_nrt_profile(None, 0)
            if rc != 0:
                raise RuntimeError(f"axon_start_nrt_profile rc={rc}")
            try:
                yield
            finally:
                n = lib.axon_stop_nrt_profile(str(output_dir).encode())
                print(f"profile: {n} file(s) -> {output_dir}", file=sys.stderr)

    mod = types.ModuleType("antenv.axon_hooks")
    mod.get_axon_ntff_profile_hook = lambda: hook
    mod.set_axon_ntff_profile_hook = lambda h: None
    sys.modules["antenv.axon_hooks"] = mod

import numpy as np
import ml_dtypes

import concourse.bacc as bacc
import concourse.bass as bass
import concourse.mybir as mybir
from concourse.bass_utils import run_bass_kernel_spmd
from concourse.tile import TileContext
from concourse.masks import make_identity

F32 = mybir.dt.float32
BF16 = mybir.dt.bfloat16
I32 = mybir.dt.int32
ALU = mybir.AluOpType
AF = mybir.ActivationFunctionType
AX = mybir.AxisListType

L = 16
GROUP = 4
D = 128
NBLK = 4                 # 128-token blocks per tile
F = NBLK * 128           # tokens per tile
EPS_RMS = 1e-8
EPS_LN = 1e-5
MAGIC = 0x5F3759DF
N_CORES = 8

_CACHE = {}


def _mkap(base, extra_off, dims):
    """Build an AP from base AP's tensor with partition dim kept and given
    free dims [[stride, count], ...] (element units)."""
    return bass.AP(tensor=base.tensor, offset=base.offset + extra_off,
                   ap=[base.ap[0]] + [list(d) for d in dims])


def _bcast(ap, n):
    """Append a stride-0 inner free dim of size n to an AP."""
    return bass.AP(tensor=ap.tensor, offset=ap.offset,
                   ap=list(ap.ap) + [[0, n]])


def _newton_rsqrt(nc, pool, x, shape, iters=2):
    """y = rsqrt(x) for x [128, *shape] f32 tile (positive). Returns y tile."""
    y = pool.tile([128] + list(shape), F32, tag="nw_y", name="nw_y")
    xi = x.bitcast(I32)
    yi = y.bitcast(I32)
    nc.vector.tensor_scalar(out=yi[:], in0=xi[:], scalar1=1, scalar2=0,
                            op0=ALU.logical_shift_right,
                            op1=ALU.logical_shift_right)
    nc.vector.tensor_scalar(out=yi[:], in0=yi[:], scalar1=-1, scalar2=MAGIC,
                            op0=ALU.mult, op1=ALU.add)
    t = pool.tile([128] + list(shape), F32, tag="nw_t", name="nw_t")
    for _ in range(iters):
        nc.vector.tensor_mul(t[:], y[:], y[:])
        nc.vector.scalar_tensor_tensor(out=t[:], in0=t[:], scalar=-0.5,
                                       in1=x[:], op0=ALU.mult, op1=ALU.mult)
        nc.vector.scalar_tensor_tensor(out=y[:], in0=t[:], scalar=1.5,
                                       in1=y[:], op0=ALU.add, op1=ALU.mult)
    return y


def build(tiles_per_core=16):
    nc = bacc.Bacc("TRN2", target_bir_lowering=False)
    n_tok = tiles_per_core * F

    embr = nc.dram_tensor("embr", [n_tok, D], BF16, kind="ExternalInput")
    embc = nc.dram_tensor("embc", [D, n_tok], BF16, kind="ExternalInput")
    # wallT1: col l (l<16) = w[l]; col 16 = ones; col 17 = ones (ssq matmul)
    wallT1 = nc.dram_tensor("wallT1", [D, L + 2], BF16, kind="ExternalInput")
    w1p = nc.dram_tensor("w1p", [D, L * 2 * 128], BF16, kind="ExternalInput")
    b1p = nc.dram_tensor("b1p", [128, 2 * L], F32, kind="ExternalInput")
    w2p = nc.dram_tensor("w2p", [128, L * 2 * D], BF16, kind="ExternalInput")
    out = nc.dram_tensor("out", [n_tok, D], F32, kind="ExternalOutput")

    embr_v = embr.rearrange("(T b p) d -> T p b d", b=NBLK, p=128)
    out_v = out.rearrange("(T b p) d -> T p b d", b=NBLK, p=128)

    NS = 4 if tiles_per_core % 4 == 0 else (
        2 if tiles_per_core % 2 == 0 else 1)

    with TileContext(nc) as tc, ExitStack() as es:
        cst = es.enter_context(tc.tile_pool(name="cst", bufs=1))
        identb = cst.tile([128, 128], BF16)
        make_identity(nc, identb[:])
        wallT1_sb = cst.tile([128, L + 2], BF16)
        nc.sync.dma_start(wallT1_sb[:], wallT1[:])
        w1p_sb = cst.tile([128, L, 2, 128], BF16)
        nc.sync.dma_start(w1p_sb[:], w1p[:].rearrange(
            "d (l h m) -> d l h m", l=L, h=2))
        b1p_sb = cst.tile([128, 2 * L], F32)
        nc.sync.dma_start(b1p_sb[:], b1p[:])
        w2p_sb = cst.tile([128, L, 2, D], BF16)
        nc.sync.dma_start(w2p_sb[:], w2p[:].rearrange(
            "m (l k d) -> m l k d", l=L, k=2))

        sp = es.enter_context(tc.tile_pool(name="state", bufs=NS))
        big = es.enter_context(tc.tile_pool(name="big", bufs=NS + 2))
        sml = es.enter_context(tc.tile_pool(name="sml", bufs=12))
        nwp = es.enter_context(tc.tile_pool(name="nw", bufs=12))
        spd = es.enter_context(tc.tile_pool(name="spd", bufs=1))
        pp_par = es.enter_context(tc.tile_pool(name="pp_par", bufs=NS,
                                               space="PSUM"))
        pp_h1 = es.enter_context(tc.tile_pool(name="pp_h1", bufs=1,
                                              space="PSUM"))
        pp_mm = es.enter_context(tc.tile_pool(name="pp_mm", bufs=1,
                                              space="PSUM"))
        pp_xt = es.enter_context(tc.tile_pool(name="pp_xt", bufs=1,
                                              space="PSUM"))

        def stat_matmuls(st, src_col, psq_col, wcols, nwide, mm_ps):
            """Stream k's 32-partition band of mm_ps:
            1) ssq matmul FIRST: nwide-col stationary ending in the ones
               column writes rows [b0 : b0+nwide); only row b0+nwide-1
               (= sum of psq = ssq) survives, since
            2) dots matmul then overwrites rows [b0 : b0+nwide-1) with
               wcols.T @ src_col (dots / rowsum)."""
            b0 = 32 * st["k"]
            wv = wallT1_sb[:]
            sel = bass.AP(tensor=wv.tensor, offset=wv.offset + (18 - nwide),
                          ap=[wv.ap[0], [1, nwide]])
            nc.tensor.matmul(mm_ps[b0:b0 + nwide, :], sel,
                             psq_col[:], start=True, stop=True,
                             tile_position=(0, b0),
                             skip_group_check=True)
            nrow = wcols.ap[-1][1]
            nc.tensor.matmul(mm_ps[b0:b0 + nrow, :], wcols,
                             src_col[:], start=True, stop=True,
                             tile_position=(0, b0),
                             skip_group_check=True)

        def transpose_stats(mm_ps, tag):
            """Stats -> row layout [128, NBLK, 128] f32 (lane 32k+j = stat_j
            of stream k)."""
            mm_sb = big.tile([128, F], BF16, tag="mm_sb", name="mm_sb")
            nc.scalar.copy(mm_sb[:], mm_ps[:])
            tt_ps = pp_xt.tile([128, F], BF16, tag="xt", name="tt_ps")
            for c in range(NBLK):
                nc.tensor.matmul(tt_ps[:, c * 128:(c + 1) * 128],
                                 mm_sb[:, c * 128:(c + 1) * 128],
                                 identb[:],
                                 is_transpose=True, start=True, stop=True,
                                 skip_group_check=True)
            row = sml.tile([128, NBLK, 128], F32, tag=tag, name=tag,
                           bufs=3)
            nc.vector.tensor_copy(
                _mkap(row[:], 0, [[1, F]]), tt_ps[:])
            return row

        def creation_finish(sts, s_idx, stats_row, sh):
            """stats_row [128, NBLK, 128] (lane 32k+j = dot_j, 32k+16 =
            rowsum, 32k+17 = ssq) -> scaled sdots_all[:, :, s_idx] and
            rs_all[:, :, s_idx]."""
            ns_ = len(sts)
            sa = sh["sdots_all"][:]
            sr = stats_row[:]
            xs = sml.tile([128, NS, NBLK], F32, tag="xs_cr", name="xs_cr")
            nc.vector.tensor_scalar(
                out=xs[:, 0:ns_],
                in0=_mkap(sr, 17, [[32, ns_], [128, NBLK]]),
                scalar1=1.0 / D, scalar2=EPS_RMS,
                op0=ALU.mult, op1=ALU.add)
            rms = _newton_rsqrt(nc, nwp, xs, (NS, NBLK))
            r_ap = rms[:]
            for k in range(ns_):
                # out: sdots_all slice (blk, l) at (k, s_idx)
                nc.vector.scalar_tensor_tensor(
                    out=_mkap(sa, k * 5 * NBLK * L + s_idx * NBLK * L,
                              [[L, NBLK], [1, L]]),
                    in0=_mkap(sr, 32 * k, [[128, NBLK], [1, L]]),
                    scalar=1.0,
                    in1=_mkap(r_ap, k * NBLK, [[1, NBLK], [0, L]]),
                    op0=ALU.bypass, op1=ALU.mult)

        def tile_start(it, k, sh):
            st = {"it": it, "k": k, "sh": sh}
            st["slots"] = sp.tile([128, 5, NBLK, D], BF16, tag="slots",
                                  name="slots")
            st["trash"] = sp.tile([128, NBLK, D], BF16, tag="trash",
                                  name="trash")
            st["partial_ps"] = pp_par.tile([128, F], F32, tag="par",
                                           name="par")
            # emb bf16 row layout (host pre-cast)
            nc.sync.dma_start(out=st["slots"][:, 0], in_=embr_v[bass.ds(it, 1)])
            # emb bf16 column layout (host pre-transposed)
            ecol = big.tile([128, F], BF16, tag="ecol", name="ecol")
            nc.sync.dma_start(out=ecol[:],
                              in_=embc[:, bass.ds(it * F, F)])
            st["ecol"] = ecol
            esq = big.tile([128, F], BF16, tag="psq", name="esq")
            nc.scalar.activation(out=esq[:], in_=ecol[:], func=AF.Square)
            st["esq"] = esq
            return st

        def emit_layer(sts, l, sh):
            ns_ = len(sts)
            g, j = l // GROUP, l % GROUP
            nsrc = g + 1
            has_p = j > 0
            n = nsrc + (1 if has_p else 0)
            last = l == L - 1
            sdots_all = sh["sdots_all"]

            # source-outermost layout [128, 5, NS, NBLK] so source slices
            # collapse to [128, n, NS*NBLK] (custom-DVE rank limit)
            SEG = NS * NBLK
            E_T = sml.tile([128, 5, NS, NBLK], F32, tag="E_T", name="E_T")
            e_ap = E_T[:]
            statics_out = bass.AP(
                tensor=e_ap.tensor, offset=e_ap.offset,
                ap=[e_ap.ap[0], [NBLK, ns_], [SEG, nsrc], [1, NBLK]])
            nc.scalar.activation(out=statics_out,
                                 in_=sdots_all[:, 0:ns_, 0:nsrc, :, l],
                                 func=AF.Tanh, scale=0.5)

            if has_p:
                pr = sh["pstat_row"][:]
                # pstat_row lanes: 32k+0 = dot, 32k+1 = rowsum, 32k+2 = ssq
                xp = sml.tile([128, NS, NBLK], F32, tag="xp", name="xp")
                nc.vector.tensor_scalar(
                    out=xp[:, 0:ns_],
                    in0=_mkap(pr, 2, [[32, ns_], [128, NBLK]]),
                    scalar1=1.0 / D, scalar2=EPS_RMS,
                    op0=ALU.mult, op1=ALU.add)
                rmsp = _newton_rsqrt(nc, nwp, xp, (NS, NBLK), iters=1)
                lp = sml.tile([128, NS, NBLK], F32, tag="lp", name="lp")
                nc.vector.tensor_mul(
                    lp[:, 0:ns_],
                    _mkap(pr, 0, [[32, ns_], [128, NBLK]]),
                    rmsp[:, 0:ns_])
                nc.scalar.activation(out=E_T[:, nsrc, 0:ns_, :],
                                     in_=lp[:, 0:ns_],
                                     func=AF.Tanh, scale=0.5)

            def seg(t, nn):
                # [128, nn, SEG] collapsed view of a [128, 5, NS, NBLK] tile
                return _mkap(t[:], 0, [[SEG, nn], [1, SEG]])

            Ev = seg(E_T, n)
            Bt = sml.tile([128, 5, NS, NBLK], F32, tag="B", name="Bt")
            nc.vector.tensor_scalar(out=seg(Bt, n), in0=Ev,
                                    scalar1=-1.0, scalar2=-1.0,
                                    op0=ALU.mult, op1=ALU.subtract)
            R = sml.tile([128, 5, NS, NBLK], F32, tag="R", name="R")
            nc.vector.reciprocal_approx_fast(seg(R, n), seg(Bt, n))
            E = sml.tile([128, 5, NS, NBLK], F32, tag="E", name="E")
            nc.vector.tensor_scalar(out=seg(E, n), in0=seg(R, n),
                                    scalar1=2.0, scalar2=-1.0,
                                    op0=ALU.mult, op1=ALU.add)
            den = sml.tile([128, NS, NBLK], F32, tag="den", name="den")
            # reduce over the (outer) source dim via AP reordering
            nc.vector.tensor_reduce(den[:, 0:ns_],
                                    _mkap(E[:], 0, [[1, SEG], [SEG, n]]),
                                    axis=AX.X, op=ALU.add)
            if last:
                # final output must be normalized: wts = E / den
                rd = sml.tile([128, NS, NBLK], F32, tag="rd", name="rd")
                nc.vector.reciprocal_approx_fast(rd[:, 0:ns_], den[:, 0:ns_])
                wts = sml.tile([128, 5, NS, NBLK], F32, tag="wts",
                               name="wts")
                nc.vector.scalar_tensor_tensor(
                    out=seg(wts, n), in0=seg(E, n),
                    scalar=1.0,
                    in1=_mkap(rd[:], 0, [[0, n], [1, SEG]]),
                    op0=ALU.bypass, op1=ALU.mult)
            else:
                # unnormalized u = sum_i E_i V_i; the 1/den normalizer is
                # folded into LayerNorm: LN(u/den) = (u - mu_u) *
                # rsqrt(var_u + eps*den^2), exactly
                wts = E

            def wsc(k, blk, i):
                # per-partition scalar AP: wts for (stream k, blk, source i)
                wv = wts[:]
                return bass.AP(tensor=wv.tensor,
                               offset=wv.offset + i * SEG + k * NBLK + blk,
                               ap=[wv.ap[0], [1, 1]])

            # weighted sum: per-block fused mult-add stt chains (DVE),
            # hsum riding the last op's hardware accumulator
            hsum = sml.tile([128, NS, NBLK], F32, tag="hsum", name="hsum")
            hs = []
            for st in sts:
                k = st["k"]
                slots = st["slots"]
                h = big.tile([128, NBLK, D], F32 if last else BF16,
                             tag="h_f32" if last else "h", name="h")
                hs.append(h)
                for blk in range(NBLK):
                    acc = hsum[:, k, blk:blk + 1] if not last else None
                    if n == 1:
                        nc.vector.tensor_scalar(
                            out=h[:, blk, :], in0=slots[:, 0, blk, :],
                            scalar1=wsc(k, blk, 0), scalar2=0.0,
                            op0=ALU.mult, op1=ALU.add, accum_out=acc)
                    else:
                        nc.vector.tensor_scalar(
                            out=h[:, blk, :], in0=slots[:, 0, blk, :],
                            scalar1=wsc(k, blk, 0), scalar2=None,
                            op0=ALU.mult)
                    for i in range(1, n):
                        nc.vector.scalar_tensor_tensor(
                            out=h[:, blk, :], in0=slots[:, i, blk, :],
                            scalar=wsc(k, blk, i),
                            in1=h[:, blk, :],
                            op0=ALU.mult, op1=ALU.add,
                            accum_out=(acc if i == n - 1 else None))
                if last:
                    nc.sync.dma_start(out=out_v[bass.ds(st["it"], 1)],
                                      in_=h[:])
            if last:
                return

            # hssq per block with accumulator rides (trash outputs)
            hssq = sml.tile([128, NS, NBLK], F32, tag="hssq", name="hssq")
            for st, h in zip(sts, hs):
                k = st["k"]
                for blk in range(NBLK):
                    nc.vector.scalar_tensor_tensor(
                        out=st["trash"][:, blk, :], in0=h[:, blk, :],
                        scalar=1.0, in1=h[:, blk, :],
                        op0=ALU.bypass, op1=ALU.mult,
                        accum_out=hssq[:, k, blk:blk + 1])

            m2 = sml.tile([128, NS, NBLK], F32, tag="m2", name="m2")
            nc.vector.tensor_mul(m2[:, 0:ns_], hsum[:, 0:ns_], hsum[:, 0:ns_])
            den2e = sml.tile([128, NS, NBLK], F32, tag="den2e",
                             name="den2e")
            nc.vector.scalar_tensor_tensor(
                out=den2e[:, 0:ns_], in0=den[:, 0:ns_], scalar=EPS_LN,
                in1=den[:, 0:ns_], op0=ALU.mult, op1=ALU.mult)
            t1 = sml.tile([128, NS, NBLK], F32, tag="t1", name="t1")
            nc.vector.scalar_tensor_tensor(
                out=t1[:, 0:ns_], in0=hssq[:, 0:ns_], scalar=1.0 / D,
                in1=den2e[:, 0:ns_], op0=ALU.mult, op1=ALU.add)
            xs2 = sml.tile([128, NS, NBLK], F32, tag="xs2", name="xs2")
            nc.vector.scalar_tensor_tensor(
                out=xs2[:, 0:ns_], in0=m2[:, 0:ns_], scalar=-1.0 / (D * D),
                in1=t1[:, 0:ns_], op0=ALU.mult, op1=ALU.add)
            s_ln = _newton_rsqrt(nc, nwp, xs2, (NS, NBLK), iters=1)
            mu = sml.tile([128, NS, NBLK], F32, tag="mu", name="mu")
            nc.vector.tensor_scalar_mul(mu[:, 0:ns_], hsum[:, 0:ns_], 1.0 / D)

            # xn = (h - mu) * s per block (fused dual-scalar ts)
            for st, h in zip(sts, hs):
                k = st["k"]
                xn = big.tile([128, NBLK, D], BF16, tag="xn", name="xn")
                for blk in range(NBLK):
                    nc.vector.tensor_scalar(
                        out=xn[:, blk, :], in0=h[:, blk, :],
                        scalar1=mu[:, k, blk:blk + 1],
                        scalar2=s_ln[:, k, blk:blk + 1],
                        op0=ALU.subtract, op1=ALU.mult)
                xnT_ps = pp_xt.tile([128, F], BF16, tag="xt", name="xnT_ps")
                for blk in range(NBLK):
                    nc.tensor.matmul(xnT_ps[:, blk * 128:(blk + 1) * 128],
                                     xn[:, blk, :], identb[:],
                                     is_transpose=True, start=True, stop=True,
                                     skip_group_check=True)
                xn_col = big.tile([128, F], BF16, tag="xn_col", name="xn_col")
                nc.vector.tensor_copy(xn_col[:], xnT_ps[:])
                st["xn_col"] = xn_col

            # MLP: W1 -> gelu -> W2 accumulating into column partial PSUM
            # (h1 double-buffered so consecutive W1 matmuls overlap gelus)
            for st in sts:
                G = []
                for half in range(2):
                    h1 = pp_h1.tile([128, F], F32, tag="h1", name="h1",
                                    bufs=2)
                    nc.tensor.matmul(h1[:], w1p_sb[:, l, half, :],
                                     st["xn_col"][:], start=True, stop=True,
                                     skip_group_check=True)
                    gh = big.tile([128, F], BF16, tag=f"g{half}", name="gh")
                    nc.scalar.activation(
                        gh[:], h1[:], AF.Gelu,
                        bias=b1p_sb[:, 2 * l + half:2 * l + half + 1])
                    G.append(gh)
                for kh in range(2):
                    nc.tensor.matmul(
                        st["partial_ps"][:], w2p_sb[:, l, kh, :], G[kh][:],
                        start=(j == 0 and kh == 0),
                        stop=((j == GROUP - 1 or l == L - 2) and kh == 1),
                        skip_group_check=True)
                pcol = big.tile([128, F], BF16, tag="pcol", name="pcol")
                nc.vector.tensor_copy(pcol[:], st["partial_ps"][:])
                st["pcol"] = pcol
                # squared partial for the ssq stats row (ACT, SBUF src to
                # avoid PSUM port contention with the pcol copy)
                psq = big.tile([128, F], BF16, tag="psq", name="psq")
                nc.scalar.activation(out=psq[:], in_=pcol[:],
                                     func=AF.Square)
                st["psq"] = psq
                # partial row slot via PE transpose + DVE copy
                prow_ps = pp_xt.tile([128, F], BF16, tag="xt",
                                     name="prow_ps")
                for blk in range(NBLK):
                    nc.tensor.matmul(prow_ps[:, blk * 128:(blk + 1) * 128],
                                     pcol[:, blk * 128:(blk + 1) * 128],
                                     identb[:],
                                     is_transpose=True, start=True, stop=True,
                                     skip_group_check=True)
                nc.vector.tensor_copy(
                    _mkap(st["slots"][:], (g + 1) * NBLK * D, [[1, F]]),
                    prow_ps[:])

            # stats for next layer (partial) or creation (commit)
            mm_ps = pp_mm.tile([128, F], F32, tag="mm", name="mm_ps")
            if j < GROUP - 1:
                wv = wallT1_sb[:]
                # cols l+1 (dot) and 16 (ones -> rowsum)
                wcols = bass.AP(tensor=wv.tensor, offset=wv.offset + l + 1,
                                ap=[wv.ap[0], [L - (l + 1), 2]])
                for st in sts:
                    stat_matmuls(st, st["pcol"], st["psq"], wcols, 3, mm_ps)
                sh["pstat_row"] = transpose_stats(mm_ps, "pstat_row")
            else:
                for st in sts:
                    stat_matmuls(st, st["pcol"], st["psq"],
                                 wallT1_sb[:, 0:L + 1], 18, mm_ps)
                stats_row = transpose_stats(mm_ps, "stats_row")
                creation_finish(sts, g + 1, stats_row, sh)

        spd_pool = spd
        with tc.For_i(0, tiles_per_core // NS, 1,
              hint_engines=(mybir.EngineType.DVE,
                            mybir.EngineType.Activation,
                            mybir.EngineType.PE,
                            mybir.EngineType.Pool)) as it0:
            sh = {}
            sh["sdots_all"] = spd_pool.tile([128, NS, 5, NBLK, L], F32,
                                            tag="sdots_all", name="sdots_all")
            sts = [tile_start(it0 * NS + k, k, sh) for k in range(NS)]
            # emb creation stats (memset clears stale psum in unused rows so
            # the transpose/selection matmuls never touch NaN garbage)
            mm_ps = pp_mm.tile([128, F], F32, tag="mm", name="mm_ps")
            nc.vector.memset(mm_ps[:], 0.0)
            for st in sts:
                stat_matmuls(st, st["ecol"], st["esq"],
                             wallT1_sb[:, 0:L + 1], 18, mm_ps)
            stats_row = transpose_stats(mm_ps, "stats_row")
            creation_finish(sts, 0, stats_row, sh)
            for l in range(L):
                emit_layer(sts, l, sh)

    nc.finalize()
    return nc


def _prep_consts(w, ln_g, ln_b, W1, b1, W2):
    bf = ml_dtypes.bfloat16
    W1p = ln_g[:, :, None] * W1                                   # diag(g) @ W1
    b1p = b1 + np.einsum("ld,ldm->lm", ln_b, W1)                  # b1 + ln_b @ W1
    w1p = np.ascontiguousarray(W1p.transpose(1, 0, 2)).reshape(D, L * 2 * 128)
    b1p_sb = b1p.reshape(L, 2, 128).transpose(2, 0, 1).reshape(128, 2 * L)
    w2p = W2.reshape(L, 2, 128, D).transpose(2, 0, 1, 3)
    w2p = np.ascontiguousarray(w2p).reshape(128, L * 2 * D)
    wallT1 = np.concatenate([w.T, np.ones((D, 2), np.float32)], axis=1)
    return {
        "wallT1": np.ascontiguousarray(wallT1).astype(bf),
        "w1p": w1p.astype(bf),
        "b1p": np.ascontiguousarray(b1p_sb).astype(np.float32),
        "w2p": w2p.astype(bf),
    }


def kernel(embedding, w, ln_g, ln_b, W1, b1, W2, b2, _tiles=16, _trace=False):
    if _trace:
        _install_ntff_hook()
    B, T, Dd = embedding.shape
    assert Dd == D
    n_tok = _tiles * F

    key = ("k", _tiles)
    if key not in _CACHE:
        _CACHE[key] = build(_tiles)
    nc = _CACHE[key]

    assert np.all(np.asarray(b2) == 0.0), "nonzero b2 unsupported"
    consts = _prep_consts(np.asarray(w, np.float32),
                          np.asarray(ln_g, np.float32),
                          np.asarray(ln_b, np.float32),
                          np.asarray(W1, np.float32),
                          np.asarray(b1, np.float32),
                          np.asarray(W2, np.float32))
    bf = ml_dtypes.bfloat16
    emb_full = np.asarray(embedding, np.float32).reshape(B * T, D)

    per_core = B * T // N_CORES
    in_maps = []
    for c in range(N_CORES):
        shard = emb_full[c * per_core:(c + 1) * per_core][:n_tok]
        shard_bf = shard.astype(bf)
        in_maps.append({"embr": shard_bf,
                        "embc": np.ascontiguousarray(shard_bf.T),
                        **consts})

    res = run_bass_kernel_spmd(nc, in_maps, core_ids=list(range(N_CORES)),
                               trace=_trace)
    outs = [res.results[c]["out"] for c in range(N_CORES)]
    full = np.stack(outs).reshape(N_CORES, n_tok, D)
    kernel.last_exec_ns = getattr(res, "exec_time_ns", None)
    kernel.last_mean_ns = getattr(res, "mean_exec_time_ns", None)
    if n_tok == per_core:
        return full.reshape(B, T, D)
    return full  # debug partial run
